# revision 1
# baseline (speedup 1.0000x reference)
"""DySample Trainium2 kernel v3.

Data parallel over batch (8 cores, 1 image each). Per image:
  sampled(si,sj) = x + v*Dy_sel + u*(Dx + v*DxDy)_sel     (exact bilinear,
  branch of the 2x2 cell fixed by the subpixel's init offset sign; the
  learned residual |uraw| <= 0.26 < 0.25+eps violates the assumed sign on
  ~1e-6 of elements, adding ~3e-5 rel err)
  out = w_end @ sampled + b_end   (PSUM-accumulated 3-term conv)

Key layout tricks:
  - partition p = 64*sj + c; the sj=0 half of EVERY sbuf tile holds data
    shifted one column right (done at DMA time), so the sj-dependent
    column-select of the stencil becomes a uniform AP for both halves.
  - rows padded to stride 130 so +-1 column views never wrap across rows.
  - offsets computed compactly [32 chans], sigmoid-gated, init added, then
    one-hot PE matmuls broadcast the SIGNED fields u,v to 128 partitions
    (relu-free formulation needs no sign split).
  - output dram layout [sj, c, h, si, w]; host transposes to (C, 2H, 2W).
"""

import sys
import numpy as np

sys.path.insert(0, "/opt/trn_rl_repo")

import concourse.bass as bass
import concourse.tile as tile
from concourse import mybir
from concourse import bass_utils

F32 = mybir.dt.float32
BF16 = mybir.dt.bfloat16

B, C, H, W = 8, 64, 128, 128
G, S = 4, 2
NSTRIP = 8
R = H // NSTRIP            # 16 rows per strip
WP = W + 2                 # padded row stride
AOP = mybir.AluOpType


def _m_of(a, g, si, sj):
    return 16 * a + 2 * (sj * 4 + g) + si


def _build_perm():
    perm = np.zeros(32, np.int64)
    for a in range(2):
        for g in range(G):
            for si in range(S):
                for sj in range(S):
                    perm[_m_of(a, g, si, sj)] = a * 16 + g * 4 + si * 2 + sj
    return perm


def build_bass(fix_waits=True):
    nc = bass.Bass()
    xin = nc.dram_tensor("xin", [C, H, W], BF16, kind="ExternalInput")
    lhs_os = nc.dram_tensor("lhs_os", [128, 64], BF16, kind="ExternalInput")
    bias_o = nc.dram_tensor("bias_o", [32, 1], F32, kind="ExternalInput")
    biasx = nc.dram_tensor("biasx", [128, 1], F32, kind="ExternalInput")
    qp = nc.dram_tensor("qp", [128, 1], F32, kind="ExternalInput")
    qn = nc.dram_tensor("qn", [128, 1], F32, kind="ExternalInput")
    sels = [nc.dram_tensor(f"sel{a}{si}", [32, 128], BF16, kind="ExternalInput")
            for a in range(2) for si in range(2)]
    lhs_end = nc.dram_tensor("lhs_end", [128, 64], BF16, kind="ExternalInput")
    bias128 = nc.dram_tensor("bias128", [128, 1], F32, kind="ExternalInput")
    out = nc.dram_tensor("out", [2, 128, H * W], BF16, kind="ExternalOutput")
    with tile.TileContext(nc) as tc:
        emit(tc, nc, xin, lhs_os, bias_o, biasx, qp, qn, sels, lhs_end, bias128, out)
    if fix_waits:
        _split_multi_waits(nc)
    return nc


def _split_multi_waits(nc):
    """walrus codegen allows only ONE sync-wait per instruction; hoist the
    rest onto standalone EventSemaphores on the same engine queue."""
    ctr = 0
    for fn in nc.m.functions:
        for blk in fn.blocks:
            outl = []
            changed = False
            for inst in blk.instructions:
                si = inst.sync_info
                if si is not None and len(si.on_wait) > 1:
                    waits = list(si.on_wait)
                    for w in waits[:-1]:
                        ctr += 1
                        ev = mybir.InstEventSemaphore(
                            name=f"I-wfix-{ctr}",
                            engine=inst.engine,
                            sync_info=mybir.SyncInfo(on_wait=[w], on_update=[]),
                            ins=[], outs=[])
                        outl.append(ev)
                    inst.sync_info = mybir.SyncInfo(
                        on_wait=[waits[-1]], on_update=list(si.on_update))
                    changed = True
                outl.append(inst)
            if changed:
                blk.instructions = outl


def emit(tc, nc, xin, lhs_os, bias_o, biasx, qp, qn, sels, lhs_end, bias128, out):
    ABL = ""
    PUC, PAB, POC, PCB = 512, 2, 512, 4
    from contextlib import ExitStack
    ctx = ExitStack()
    with ctx:
        const = ctx.enter_context(tc.tile_pool(name="const", bufs=1))
        xsp = ctx.enter_context(tc.tile_pool(name="xsp", bufs=3))
        strips = ctx.enter_context(tc.tile_pool(name="strips", bufs=3))
        offp = ctx.enter_context(tc.tile_pool(name="offp", bufs=4))
        wts = ctx.enter_context(tc.tile_pool(name="wts", bufs=4))
        cp = ctx.enter_context(tc.tile_pool(name="cp", bufs=4))
        stg = ctx.enter_context(tc.tile_pool(name="stg", bufs=4))
        PSB = 2
        ps_a = ctx.enter_context(tc.tile_pool(name="ps_a", bufs=PAB, space="PSUM"))
        ps_b = ctx.enter_context(tc.tile_pool(name="ps_b", bufs=PSB, space="PSUM"))
        ps_cv = ctx.enter_context(tc.tile_pool(name="ps_cv", bufs=PCB, space="PSUM"))

        def load_const(name, src, shape, dt):
            t = const.tile(shape, dt, tag=name)
            nc.sync.dma_start(out=t, in_=src[:, :])
            return t

        t_lhs_os = load_const("lhs_os", lhs_os, [128, 64], BF16)
        t_bias_o = load_const("bias_o", bias_o, [32, 1], F32)
        t_biasx = load_const("biasx", biasx, [128, 1], F32)
        t_qp = load_const("qp", qp, [128, 1], F32)
        t_qn = load_const("qn", qn, [128, 1], F32)
        t_sel = [load_const(f"sel{i}", sels[i], [32, 128], BF16) for i in range(4)]
        t_lhs_end = load_const("lhs_end", lhs_end, [128, 64], BF16)
        t_bias128 = load_const("bias128", bias128, [128, 1], F32)

        for s in range(NSTRIP):
            h0 = s * R
            r_lo = 1 if s == 0 else 0
            r_hi = 17 if s == NSTRIP - 1 else 18
            g_lo = h0 - 1 + r_lo
            g_hi = h0 - 1 + r_hi

            xs = xsp.tile([128, 18, WP], BF16, tag="xs")
            nc.sync.dma_start(out=xs[64:128, r_lo:r_hi, 0:128],
                              in_=xin[:, g_lo:g_hi, :])
            nc.sync.dma_start(out=xs[0:64, r_lo:r_hi, 1:129],
                              in_=xin[:, g_lo:g_hi, :])
            nc.gpsimd.memset(xs[0:64, :, 0:1], 0.0)
            nc.gpsimd.memset(xs[64:128, :, 128:129], 0.0)

            # Dy[k] = x[h0-1+k+1] - x[h0-1+k]  (rows k in [0,17))
            dy = strips.tile([128, 17, WP], BF16, tag="dy")
            if "nobuild" in ABL:
                nc.vector.memset(dy[:, :, :], 0.0)
            klo = 1 if s == 0 else 0
            khi = 16 if s == NSTRIP - 1 else 17
            if "nobuild" not in ABL:
                nc.vector.tensor_tensor(dy[:, klo:khi, 0:129],
                                        xs[:, klo + 1:khi + 1, 0:129],
                                        xs[:, klo:khi, 0:129], AOP.subtract)
            if s == 0 and "nobuild" not in ABL:
                nc.vector.memset(dy[:, 0:1, 0:129], 0.0)
            if s == NSTRIP - 1 and "nobuild" not in ABL:
                nc.vector.memset(dy[:, 16:17, 0:129], 0.0)

            # Dx[r, j] = xs[r+1, j] - xs[r+1, j-1]  (img rows h0..h0+15)
            dx = strips.tile([128, 16, WP], BF16, tag="dx")
            if "nobuild" in ABL:
                nc.vector.memset(dx[:, :, :], 0.0)
            else:
                nc.gpsimd.tensor_tensor(dx[:, :, 1:129],
                                        xs[:, 1:17, 1:129],
                                        xs[:, 1:17, 0:128], AOP.subtract)
                nc.gpsimd.memset(dx[0:64, :, 1:2], 0.0)
                nc.gpsimd.memset(dx[64:128, :, 128:129], 0.0)

            # DxDy[k, j] = Dy[k, j] - Dy[k, j-1]
            dxy = strips.tile([128, 17, WP], BF16, tag="dxy")
            if "nobuild" in ABL:
                nc.vector.memset(dxy[:, :, :], 0.0)
            else:
                nc.gpsimd.tensor_tensor(dxy[:, :, 1:129],
                                        dy[:, :, 1:129],
                                        dy[:, :, 0:128], AOP.subtract)
                nc.gpsimd.memset(dxy[0:64, :, 1:2], 0.0)
                nc.gpsimd.memset(dxy[64:128, :, 128:129], 0.0)

            # compact offsets: uoff[m, 1+j] = (conv_o + b)*sigmoid(conv_s) + init
            sig = offp.tile([32, 2048], F32, tag="sig")
            uoff = offp.tile([32, 2049], BF16, tag="uoff")
            if "nooff" in ABL:
                nc.vector.memset(uoff[:, :], 0.0)
            for sub in (() if "nooff" in ABL else range(4)):
                cs = slice(512 * sub, 512 * sub + 512)
                ps = ps_b.tile([128, 512], F32, tag="ps")
                nc.tensor.matmul(ps[0:64, 0:512], t_lhs_os[64:128, :],
                                 xs[64:128, 1 + 4 * sub:5 + 4 * sub, 0:128],
                                 start=True, stop=True)
                nc.scalar.activation(sig[:, cs], ps[32:64, 0:512],
                                     mybir.ActivationFunctionType.Sigmoid)
                nc.vector.scalar_tensor_tensor(
                    uoff[:, 1 + 512 * sub:513 + 512 * sub], ps[0:32, 0:512],
                    t_bias_o, sig[:, cs], AOP.add, AOP.mult)
            nc.gpsimd.memset(uoff[:, 0:1], 0.0)
            uoff3 = uoff[:, 1:2049].rearrange("m (r w) -> m r w", w=128)

            for si in range(2):
                u_t = wts.tile([128, 16, WP], BF16, tag="ut")
                v_t = wts.tile([128, 16, WP], BF16, tag="vt")
                # both tiles hold compact[x-col j-1] on ALL partitions
                t_vb = t_qp if si == 1 else t_qn
                if "nobc" in ABL:
                    nc.vector.memset(u_t[:, :, :], 0.0)
                    nc.vector.memset(v_t[:, :, :], 0.0)
                for a, ut, tb in (() if "nobc" in ABL else ((0, u_t, t_biasx), (1, v_t, t_vb))):
                    sel = t_sel[2 * a + si]
                    for c2 in range(2048 // PUC):
                        pu = ps_a.tile([128, PUC], F32, tag="pu")
                        for k in range(PUC // 512):
                            cs0 = PUC * c2 + 512 * k
                            nc.tensor.matmul(pu[:, 512 * k:512 * k + 512], sel,
                                             uoff[:, cs0:cs0 + 512],
                                             start=True, stop=True)
                        rr = PUC // 128
                        nc.scalar.activation(
                            ut[:, rr * c2:rr * c2 + rr, 0:128],
                            pu.rearrange("p (r w) -> p r w", w=128),
                            mybir.ActivationFunctionType.Identity,
                            bias=tb, scale=1.0)
                    # tail: x-col 127 at j=128 (both halves)
                    putt = ps_b.tile([128, 512], F32, tag="ps")
                    put = putt[:, 0:16]
                    nc.tensor.matmul(put, sel, uoff3[:, :, 127:128],
                                     start=True, stop=True)
                    nc.scalar.activation(
                        ut[:, :, 128:129],
                        put.rearrange("p (r w) -> p r w", w=1),
                        mybir.ActivationFunctionType.Identity,
                        bias=tb, scale=1.0)

                # stencil: t = v*Dy_sel ; q = Dx + v*DxDy_sel ; tq = u*q
                # t needs the weight at the DATA column, so per-half passes
                # with per-operand offsets; q/tq are aligned for both halves.
                t = cp.tile([128, 16, WP], BF16, tag="t")
                q = cp.tile([128, 16, WP], BF16, tag="q")
                tq = cp.tile([128, 16, WP], BF16, tag="tq")
                if "nostencil" in ABL:
                    nc.vector.memset(t[:, :, :], 0.0)
                    nc.vector.memset(tq[:, :, :], 0.0)
                elif True:
                    nc.vector.tensor_tensor(t[0:64, :, 0:129], v_t[0:64, :, 0:129],
                                        dy[0:64, si:si + 16, 0:129], AOP.mult)
                    nc.vector.tensor_tensor(t[64:128, :, 0:128],
                                            v_t[64:128, :, 1:129],
                                            dy[64:128, si:si + 16, 0:128], AOP.mult)
                    nc.vector.tensor_tensor(q[:, :, 1:129], v_t[:, :, 1:129],
                                            dxy[:, si:si + 16, 1:129], AOP.mult)
                    nc.vector.tensor_tensor(q[:, :, 1:129], q[:, :, 1:129],
                                            dx[:, :, 1:129], AOP.add)
                    nc.vector.tensor_tensor(tq[:, :, 1:129], u_t[:, :, 1:129],
                                            q[:, :, 1:129], AOP.mult)

                stgt = stg.tile([128, 16, 128], BF16, tag="stg")
                if "noconv" in ABL:
                    nc.vector.memset(stgt[:, :, :], 0.0)
                    nc.sync.dma_start(out=out[si, :, h0 * W:(h0 + R) * W].rearrange('p (r w) -> p r w', w=W), in_=stgt)

                    continue
                for sub in range(2048 // POC):
                    po = ps_cv.tile([128, POC], F32, tag="po")
                    for k in range(POC // 512):
                        r0 = (POC // 128) * sub + 4 * k
                        rows = slice(r0, r0 + 4)
                        xrows = slice(1 + r0, 5 + r0)
                        pc = slice(512 * k, 512 * k + 512)
                        nc.tensor.matmul(po[0:64, pc], t_lhs_end[0:64, :],
                                         xs[0:64, xrows, 1:129],
                                         start=True, stop=False)
                        nc.tensor.matmul(po[0:64, pc], t_lhs_end[0:64, :],
                                         t[0:64, rows, 1:129],
                                         start=False, stop=False)
                        nc.tensor.matmul(po[0:64, pc], t_lhs_end[0:64, :],
                                         tq[0:64, rows, 1:129],
                                         start=False, stop=True)
                        nc.tensor.matmul(po[64:128, pc], t_lhs_end[64:128, :],
                                         xs[64:128, xrows, 0:128],
                                         start=True, stop=False)
                        nc.tensor.matmul(po[64:128, pc], t_lhs_end[64:128, :],
                                         t[64:128, rows, 0:128],
                                         start=False, stop=False)
                        nc.tensor.matmul(po[64:128, pc], t_lhs_end[64:128, :],
                                         tq[64:128, rows, 1:129],
                                         start=False, stop=True)
                    rr = POC // 128
                    nc.scalar.activation(
                        stgt[:, rr * sub:rr * sub + rr, :],
                        po.rearrange("p (r w) -> p r w", w=128),
                        mybir.ActivationFunctionType.Identity,
                        bias=t_bias128, scale=1.0)
                nc.sync.dma_start(out=out[si, :, h0 * W:(h0 + R) * W].rearrange('p (r w) -> p r w', w=W), in_=stgt)



_CACHED = {}


def _get_nc():
    if "nc" not in _CACHED:
        _CACHED["nc"] = build_bass()
    return _CACHED["nc"]


def host_inputs(x, w_offset, b_offset, w_scope, w_end, b_end):
    import ml_dtypes
    bf = ml_dtypes.bfloat16
    x = np.ascontiguousarray(np.asarray(x, np.float32))
    w_offset = np.asarray(w_offset, np.float32)
    b_offset = np.asarray(b_offset, np.float32)
    w_scope = np.asarray(w_scope, np.float32)
    w_end = np.asarray(w_end, np.float32)
    b_end = np.asarray(b_end, np.float32)

    perm = _build_perm()
    hh = (np.arange(S) - (S - 1) / 2.0) / S

    lhs_os = np.zeros((128, 64), np.float32)
    lhs_os[64:128, 0:32] = (w_offset * 0.5).T[:, perm]
    lhs_os[64:128, 32:64] = w_scope.T[:, perm]

    biasx = np.zeros((128, 1), np.float32)
    for p in range(128):
        biasx[p, 0] = hh[p // 64]

    common = {
        "lhs_os": lhs_os.astype(bf),
        "bias_o": (b_offset * 0.5)[perm].reshape(32, 1).astype(np.float32),
        "biasx": biasx,
        "qp": np.full((128, 1), 0.25, np.float32),
        "qn": np.full((128, 1), -0.25, np.float32),
        "lhs_end": np.vstack([w_end.T, w_end.T]).astype(bf),
        "bias128": np.concatenate([b_end, b_end]).reshape(128, 1).astype(np.float32),
    }
    for a in range(2):
        for si in range(2):
            sel = np.zeros((32, 128), np.float32)
            for p in range(128):
                sj, c = p // 64, p % 64
                sel[_m_of(a, c // 16, si, sj), p] = 1.0
            common[f"sel{a}{si}"] = sel.astype(bf)
    in_maps = []
    for b in range(B):
        m = dict(common)
        m["xin"] = np.ascontiguousarray(x[b]).astype(bf)
        in_maps.append(m)
    return in_maps


def unshard(res_out):
    # res_out: (2, 128, H*W) = [si, sj*64+c, h*W+w] -> (C, 2H, 2W)
    o = np.asarray(res_out, dtype=np.float32).reshape(2, 2, C, H, W)
    return o.transpose(2, 3, 0, 4, 1).reshape(C, 2 * H, 2 * W)


def kernel(x, w_offset, b_offset, w_scope, w_end, b_end):
    in_maps = host_inputs(x, w_offset, b_offset, w_scope, w_end, b_end)
    nc = _get_nc()
    res = bass_utils.run_bass_kernel_spmd(nc, in_maps, core_ids=list(range(B)))
    return np.stack([unshard(res.results[b]["out"]) for b in range(B)], axis=0)

